# revision 1
# baseline (speedup 1.0000x reference)
"""Trainium2 Bass kernel for nn_PointSetAnchorPoseHead (NMS pose decode).

Runs on 8 NeuronCores via run_bass_kernel_spmd. See bottom for host glue.

Algorithm (per core, SPMD):
  heat stage: rows sharded 64/core (+2 halo). 5x5 maxpool via shifted-max
  cascades on (col,row)-in-free layout; exact key packing: for values v>t
  (t=1-2^-11) key = (v-t)*2^35 + (2047-slabidx), a 24-bit exact f32 int that
  orders by (value, then lower slabidx) and is duplicate-free within a slab.
  max8 per (joint, 25-col slab) -> per-core top-16 -> AllGather(8x16) ->
  replicated merge via max8+match_replace -> top-30 keys/joint. Owner cores
  eq-match keys to recover flat positions and sparse-gather offsets from
  their offset shard; AllReduce combines (gpos, offx, offy).
  pose stage: 98 tiles of 128 poses. score = |c|^2 - 2 q.c in one fp32 PE
  matmul (block-diag W built on device), segmented min over k, one-hot =
  (score < min + 8), PE transpose (bf16) + 3-way bf16-split gather matmul
  produces selected coords + per-joint match count. Host recomputes the few
  count!=1 sites exactly (reference f32 arithmetic).
"""

import numpy as np

J = 17
K = 30
H = 512
W = 512
NCORES = 8
RPC = H // NCORES          # 64 rows per core
PT = 128
NT = 98
NPAD = PT * NT             # 12544
CAUG = 36                  # x17, y17, 1, 0
JK = J * K                 # 510
JKP = 512
SW = 25                    # slab width; 21 slabs
NSLAB = 21
SLABW = SW + 4             # stored cols (2 halo each side, 29)
RW = RPC + 4               # stored rows (68)
SLABF = SW * RPC           # 1600 owned cells
SPT = 7                    # slabs per heat partition-tile
NTILE_H = 3
THRESH_T = float(1.0 - 2.0 ** -11)
KEYSCALE = float(2.0 ** 35)
SCORE_THRESH = 32.0
NEG = -1.0e30
OFFSZ = J * 2 * RPC * W

_CACHE = {}
LAST_EXEC_NS = None


# --------------------------------------------------------------------------
# device program
# --------------------------------------------------------------------------
def _build_program(stride, debug=False, ntiles=NT):
    import concourse.bass as bass
    import concourse.bacc as bacc
    import concourse.mybir as mybir
    from concourse import tile

    dt = mybir.dt
    Alu = mybir.AluOpType
    Ax = mybir.AxisListType
    nc = bacc.Bacc(None)

    def din(name, shape, dtype=dt.float32):
        return nc.declare_dram_parameter(name, list(shape), dtype, isOutput=False)

    poses_d = din("poses", [NPAD, CAUG])
    heat_d = din("heat", [NTILE_H * PT, SLABW * RW])
    offs_d = din("offs", [OFFSZ, 1])
    cconst_d = din("coreconst", [J, 2])
    identf_d = din("identf", [PT, PT])
    identb_d = din("identb", [PT, PT], dt.bfloat16)
    rev_d = din("revconst", [PT, SLABF])
    mtx_d = din("maskTx", [PT, 4 * 51])
    mty_d = din("maskTy", [PT, 4 * 51])
    mtc_d = din("maskTc", [PT, 4 * 51])
    mw_d = din("maskW", [J, JKP])
    jsel_d = din("jsel", [PT, 8 * J])
    msel_d = din("msel", [PT, 8 * 60])
    cgidx_d = din("cgidx", [J, NSLAB * 8])

    out_d = nc.declare_dram_parameter("out", [NPAD, 51], dt.float32, isOutput=True)
    cand_d = nc.declare_dram_parameter("cand", [J, PT], dt.float32, isOutput=True)
    if debug:
        dbg_d = nc.declare_dram_parameter("dbg", [J, 600], dt.float32, isOutput=True)

    with tile.TileContext(nc) as tc:
        with (
            tc.tile_pool(name="const", bufs=1) as cpool,
            tc.tile_pool(name="heatp", bufs=2) as hpool,
            tc.tile_pool(name="work", bufs=1) as wpool,
            tc.tile_pool(name="small", bufs=1) as spool,
            tc.tile_pool(name="pose", bufs=1) as ppool,
            tc.tile_pool(name="loop", bufs=3) as lpool,
            tc.tile_pool(name="psA", bufs=2, space="PSUM") as psA,
            tc.tile_pool(name="psB", bufs=1, space="PSUM") as psB,
            tc.tile_pool(name="dram", bufs=1, space="DRAM") as dpool,
        ):
            # ---------- constants ----------
            identf = cpool.tile([PT, PT], dt.float32)
            nc.sync.dma_start(identf[:], identf_d[:])
            identb = cpool.tile([PT, PT], dt.bfloat16)
            nc.sync.dma_start(identb[:], identb_d[:])
            rev = cpool.tile([PT, SLABF], dt.float32)
            nc.sync.dma_start(rev[:], rev_d[:])
            mtx = cpool.tile([PT, 4 * 51], dt.float32)
            nc.sync.dma_start(mtx[:], mtx_d[:])
            mty = cpool.tile([PT, 4 * 51], dt.float32)
            nc.sync.dma_start(mty[:], mty_d[:])
            mtc = cpool.tile([PT, 4 * 51], dt.float32)
            nc.sync.dma_start(mtc[:], mtc_d[:])
            mw = cpool.tile([J, JKP], dt.float32)
            nc.sync.dma_start(mw[:], mw_d[:])
            jsel = cpool.tile([PT, 8 * J], dt.float32)
            nc.sync.dma_start(jsel[:], jsel_d[:])
            msel = cpool.tile([PT, 8 * 60], dt.float32)
            nc.sync.dma_start(msel[:], msel_d[:])
            cconst = cpool.tile([J, 2], dt.float32)
            nc.sync.dma_start(cconst[:], cconst_d[:])
            cg_f = cpool.tile([J, NSLAB * 8], dt.float32)
            nc.sync.dma_start(cg_f[:], cgidx_d[:])

            posesb0 = ppool.tile([PT, NT * CAUG], dt.float32)
            nc.sync.dma_start(
                posesb0[:],
                bass.AP(poses_d[:].tensor, 0,
                        [[CAUG, PT], [PT * CAUG, NT], [1, CAUG]]))
            posesb = ppool.tile([PT, NT * CAUG], dt.float32)
            nc.vector.tensor_copy(posesb[:], posesb0[:])
            identfc = cpool.tile([PT, PT], dt.float32)
            nc.vector.tensor_copy(identfc[:], identf[:])
            identbc = cpool.tile([PT, PT], dt.bfloat16)
            nc.vector.tensor_copy(identbc[:], identb[:])
            jselc = cpool.tile([PT, 8 * J], dt.float32)
            nc.vector.tensor_copy(jselc[:], jsel[:])

            # ---------- heat stage ----------
            kall = spool.tile([J, NSLAB * 8], dt.float32)
            for ti in range(NTILE_H):
                hx = hpool.tile([PT, SLABW * RW], dt.float32, tag="heat")
                nc.sync.dma_start(hx[:], heat_d[ti * PT:(ti + 1) * PT, :])

                def ap(t, coff, roff, ccnt, rcnt, rw):
                    return bass.AP(t.tensor, coff * rw + roff,
                                   [[t.shape[1], PT], [rw, ccnt], [1, rcnt]])

                m1 = wpool.tile([PT, SLABW * 67], dt.float32, tag="m1")
                nc.vector.tensor_tensor(out=ap(m1, 0, 0, SLABW, 67, 67),
                                        in0=ap(hx, 0, 0, SLABW, 67, RW),
                                        in1=ap(hx, 0, 1, SLABW, 67, RW), op=Alu.max)
                m2 = wpool.tile([PT, SLABW * 65], dt.float32, tag="m2")
                nc.vector.tensor_tensor(out=ap(m2, 0, 0, SLABW, 65, 65),
                                        in0=ap(m1, 0, 0, SLABW, 65, 67),
                                        in1=ap(m1, 0, 2, SLABW, 65, 67), op=Alu.max)
                w5r = wpool.tile([PT, SLABW * RPC], dt.float32, tag="w5r")
                nc.vector.tensor_tensor(out=ap(w5r, 0, 0, SLABW, RPC, RPC),
                                        in0=ap(m2, 0, 0, SLABW, RPC, 65),
                                        in1=ap(hx, 0, 4, SLABW, RPC, RW), op=Alu.max)
                n1 = wpool.tile([PT, 28 * RPC], dt.float32, tag="n1")
                nc.vector.tensor_tensor(out=ap(n1, 0, 0, 28, RPC, RPC),
                                        in0=ap(w5r, 0, 0, 28, RPC, RPC),
                                        in1=ap(w5r, 1, 0, 28, RPC, RPC), op=Alu.max)
                n2 = wpool.tile([PT, 26 * RPC], dt.float32, tag="n2")
                nc.vector.tensor_tensor(out=ap(n2, 0, 0, 26, RPC, RPC),
                                        in0=ap(n1, 0, 0, 26, RPC, RPC),
                                        in1=ap(n1, 2, 0, 26, RPC, RPC), op=Alu.max)
                w55 = wpool.tile([PT, SW * RPC], dt.float32, tag="w55")
                nc.vector.tensor_tensor(out=ap(w55, 0, 0, SW, RPC, RPC),
                                        in0=ap(n2, 0, 0, SW, RPC, RPC),
                                        in1=ap(w5r, 4, 0, SW, RPC, RPC), op=Alu.max)
                eq = wpool.tile([PT, SW * RPC], dt.float32, tag="eq")
                nc.vector.tensor_tensor(out=ap(eq, 0, 0, SW, RPC, RPC),
                                        in0=ap(hx, 2, 2, SW, RPC, RW),
                                        in1=ap(w55, 0, 0, SW, RPC, RPC),
                                        op=Alu.is_equal)
                r1 = wpool.tile([PT, SW * RPC], dt.float32, tag="r1")
                nc.vector.scalar_tensor_tensor(
                    out=ap(r1, 0, 0, SW, RPC, RPC),
                    in0=ap(hx, 2, 2, SW, RPC, RW), scalar=-THRESH_T,
                    in1=ap(eq, 0, 0, SW, RPC, RPC), op0=Alu.add, op1=Alu.mult)
                keyt = wpool.tile([PT, SLABF], dt.float32, tag="keyt")
                nc.vector.scalar_tensor_tensor(
                    out=keyt[:], in0=r1[:], scalar=KEYSCALE, in1=rev[:],
                    op0=Alu.mult, op1=Alu.add)
                k8t = wpool.tile([PT, 8], dt.float32, tag="k8t")
                nc.vector.max(k8t[:], keyt[:])
                for cgl in range(SPT):
                    nc.sync.dma_start(
                        kall[:, (ti * SPT + cgl) * 8:(ti * SPT + cgl + 1) * 8],
                        k8t[cgl * J:cgl * J + J, :])

            # per-core top-16
            kwork = spool.tile([J, NSLAB * 8], dt.float32)
            nc.vector.tensor_copy(kwork[:], kall[:])
            key16 = spool.tile([J, 16], dt.float32)
            nc.vector.max(key16[:, 0:8], kwork[:])
            nc.vector.match_replace(kwork[:], key16[:, 0:8], kwork[:], NEG)
            nc.vector.max(key16[:, 8:16], kwork[:])

            # decode all local per-slab candidates -> gposall [17, 168]
            ki = spool.tile([J, NSLAB * 8], dt.int32)
            kclamp = spool.tile([J, NSLAB * 8], dt.float32)
            nc.vector.tensor_scalar_max(kclamp[:], kall[:], 0.0)
            nc.vector.tensor_copy(ki[:], kclamp[:])
            s11 = spool.tile([J, NSLAB * 8], dt.int32)
            nc.vector.tensor_scalar(out=s11[:], in0=ki[:], scalar1=2047,
                                    scalar2=None, op0=Alu.bitwise_and)
            nc.vector.tensor_scalar(out=s11[:], in0=s11[:], scalar1=-2047,
                                    scalar2=-1, op0=Alu.add, op1=Alu.mult)
            ci = spool.tile([J, NSLAB * 8], dt.int32)
            nc.vector.tensor_scalar(out=ci[:], in0=s11[:], scalar1=6,
                                    scalar2=None, op0=Alu.arith_shift_right)
            ri = spool.tile([J, NSLAB * 8], dt.int32)
            nc.vector.tensor_scalar(out=ri[:], in0=s11[:], scalar1=RPC - 1,
                                    scalar2=None, op0=Alu.bitwise_and)
            cf = spool.tile([J, NSLAB * 8], dt.float32)
            nc.vector.tensor_copy(cf[:], ci[:])
            rf = spool.tile([J, NSLAB * 8], dt.float32)
            nc.vector.tensor_copy(rf[:], ri[:])
            gcol = spool.tile([J, NSLAB * 8], dt.float32)
            nc.vector.scalar_tensor_tensor(out=gcol[:], in0=cg_f[:], scalar=float(SW),
                                           in1=cf[:], op0=Alu.mult, op1=Alu.add)
            gposall = spool.tile([J, NSLAB * 8], dt.float32)
            nc.vector.scalar_tensor_tensor(out=gposall[:], in0=rf[:], scalar=float(W),
                                           in1=gcol[:], op0=Alu.mult, op1=Alu.add)
            nc.vector.tensor_scalar(out=gposall[:], in0=gposall[:],
                                    scalar1=cconst[:, 0:1], scalar2=None, op0=Alu.add)

            # ---------- AllGather keys ----------
            ag_in = dpool.tile([J, 16], dt.float32)
            ag_out = dpool.tile([NCORES * J, 16], dt.float32)
            nc.sync.dma_start(ag_in[:], key16[:])
            nc.gpsimd.collective_compute(
                "AllGather", Alu.bypass,
                replica_groups=[list(range(NCORES))],
                ins=[ag_in[:]], outs=[ag_out[:]])
            kpool2 = spool.tile([J, NCORES * 16], dt.float32)
            for c in range(NCORES):
                nc.sync.dma_start(kpool2[:, c * 16:(c + 1) * 16],
                                  ag_out[c * J:(c + 1) * J, :])

            # merge: top-32 keys (use first 30)
            kmw = spool.tile([J, NCORES * 16], dt.float32)
            nc.vector.tensor_copy(kmw[:], kpool2[:])
            fkeys = spool.tile([J, 32], dt.float32)
            for r in range(4):
                nc.vector.max(fkeys[:, r * 8:(r + 1) * 8], kmw[:])
                if r < 3:
                    nc.vector.match_replace(kmw[:], fkeys[:, r * 8:(r + 1) * 8],
                                            kmw[:], NEG)

            # eq-match final keys against local candidates -> local gpos contrib
            eq3 = spool.tile([J, K * NSLAB * 8], dt.float32)
            fk_b = bass.AP(fkeys.tensor, 0, [[32, J], [1, K], [0, NSLAB * 8]])
            ka_b = bass.AP(kall.tensor, 0,
                           [[NSLAB * 8, J], [0, K], [1, NSLAB * 8]])
            e3 = eq3[:].rearrange("j (m i) -> j m i", m=K)
            nc.vector.tensor_tensor(out=e3, in0=fk_b, in1=ka_b, op=Alu.is_equal)
            gp_b = bass.AP(gposall.tensor, 0,
                           [[NSLAB * 8, J], [0, K], [1, NSLAB * 8]])
            nc.vector.tensor_tensor(out=e3, in0=e3, in1=gp_b, op=Alu.mult)
            gposc = spool.tile([J, K], dt.float32)
            nc.vector.tensor_reduce(gposc[:], e3, axis=Ax.X, op=Alu.add)
            validm = spool.tile([J, K], dt.float32)
            nc.vector.tensor_scalar(out=validm[:], in0=gposc[:], scalar1=0.5,
                                    scalar2=None, op0=Alu.is_gt)

            # local offsets gather (only for candidates we own)
            gq = spool.tile([J, K], dt.int32)
            nc.vector.tensor_copy(gq[:], gposc[:])
            gqr = spool.tile([J, K], dt.int32)
            nc.vector.tensor_scalar(out=gqr[:], in0=gq[:], scalar1=9,
                                    scalar2=None, op0=Alu.arith_shift_right)
            rowf = spool.tile([J, K], dt.float32)
            nc.vector.tensor_copy(rowf[:], gqr[:])
            gqc = spool.tile([J, K], dt.int32)
            nc.vector.tensor_scalar(out=gqc[:], in0=gq[:], scalar1=W - 1,
                                    scalar2=None, op0=Alu.bitwise_and)
            colf = spool.tile([J, K], dt.float32)
            nc.vector.tensor_copy(colf[:], gqc[:])
            lrow = spool.tile([J, K], dt.float32)
            nc.vector.tensor_scalar(out=lrow[:], in0=rowf[:],
                                    scalar1=cconst[:, 1:2], scalar2=None,
                                    op0=Alu.subtract)
            nc.vector.tensor_scalar(out=lrow[:], in0=lrow[:], scalar1=0.0,
                                    scalar2=float(RPC - 1), op0=Alu.max, op1=Alu.min)
            jrowf = spool.tile([J, 1], dt.float32)
            jr_i = spool.tile([J, 1], dt.int32)
            nc.gpsimd.iota(jr_i[:], pattern=[[0, 1]], base=0, channel_multiplier=1)
            nc.vector.tensor_copy(jrowf[:], jr_i[:])
            j2 = spool.tile([J, 1], dt.float32)
            nc.vector.tensor_scalar_mul(j2[:], jrowf[:], float(2 * RPC * W))
            idx_x = spool.tile([J, K], dt.float32)
            nc.vector.scalar_tensor_tensor(out=idx_x[:], in0=lrow[:], scalar=float(W),
                                           in1=colf[:], op0=Alu.mult, op1=Alu.add)
            nc.vector.tensor_scalar(out=idx_x[:], in0=idx_x[:],
                                    scalar1=j2[:, 0:1], scalar2=None, op0=Alu.add)
            idx_y = spool.tile([J, K], dt.float32)
            nc.vector.tensor_scalar(out=idx_y[:], in0=idx_x[:],
                                    scalar1=float(RPC * W), scalar2=None, op0=Alu.add)
            # stage [1, 1024] then repack p-major into [128, 8]
            stage = spool.tile([1, 1024], dt.float32)
            nc.vector.memset(stage[:], 0.0)
            nc.sync.dma_start(bass.AP(stage.tensor, 0, [[1024, 1], [1, JK]]),
                              idx_x[:])
            nc.sync.dma_start(bass.AP(stage.tensor, 512, [[1024, 1], [1, JK]]),
                              idx_y[:])
            idxp_f = spool.tile([PT, 8], dt.float32)
            nc.sync.dma_start(idxp_f[:],
                              bass.AP(stage.tensor, 0, [[1024, 1], [1, 1024]]))
            idxp = spool.tile([PT, 8], dt.int32)
            nc.vector.tensor_copy(idxp[:], idxp_f[:])
            offs_g = spool.tile([PT, 8], dt.float32)
            nc.vector.memset(offs_g[:], 0.0)
            for c in range(8):
                nc.gpsimd.indirect_dma_start(
                    out=offs_g[:, c:c + 1], out_offset=None,
                    in_=offs_d[:],
                    in_offset=bass.IndirectOffsetOnAxis(ap=idxp[:, c:c + 1], axis=0),
                    bounds_check=OFFSZ - 1, oob_is_err=False)
            rhs_t = spool.tile([PT, 8 * 60], dt.float32)
            for c in range(8):
                ob = bass.AP(offs_g.tensor, c, [[8, PT], [0, 60]])
                nc.vector.tensor_tensor(out=rhs_t[:, c * 60:(c + 1) * 60],
                                        in0=ob, in1=msel[:, c * 60:(c + 1) * 60],
                                        op=Alu.mult)
            offps = psB.tile([J, 60], dt.float32, tag="psb")
            for c in range(8):
                nc.tensor.matmul(offps[:], jselc[:, c * J:(c + 1) * J],
                                 rhs_t[:, c * 60:(c + 1) * 60],
                                 start=(c == 0), stop=(c == 7))
            offc = spool.tile([J, 60], dt.float32)
            nc.vector.tensor_copy(offc[:], offps[:])
            nc.vector.tensor_tensor(out=offc[:, 0:K], in0=offc[:, 0:K],
                                    in1=validm[:], op=Alu.mult)
            nc.vector.tensor_tensor(out=offc[:, K:60], in0=offc[:, K:60],
                                    in1=validm[:], op=Alu.mult)
            gpos_m = spool.tile([J, K], dt.float32)
            nc.vector.tensor_tensor(out=gpos_m[:], in0=gposc[:], in1=validm[:],
                                    op=Alu.mult)

            # ---------- AllReduce (gpos | offx | offy) ----------
            ar_in = dpool.tile([J, 90], dt.float32)
            ar_out = dpool.tile([J, 90], dt.float32)
            nc.sync.dma_start(ar_in[:, 0:K], gpos_m[:])
            nc.sync.dma_start(ar_in[:, K:90], offc[:])
            nc.gpsimd.collective_compute(
                "AllReduce", Alu.add,
                replica_groups=[list(range(NCORES))],
                ins=[ar_in[:]], outs=[ar_out[:]])
            agg = spool.tile([J, 90], dt.float32)
            nc.sync.dma_start(agg[:], ar_out[:])

            # final candidate coords (reference arithmetic: stride*(x+off))
            yq = spool.tile([J, K], dt.int32)
            nc.vector.tensor_copy(yq[:], agg[:, 0:K])
            yqs = spool.tile([J, K], dt.int32)
            nc.vector.tensor_scalar(out=yqs[:], in0=yq[:], scalar1=9,
                                    scalar2=None, op0=Alu.arith_shift_right)
            yf = spool.tile([J, K], dt.float32)
            nc.vector.tensor_copy(yf[:], yqs[:])
            xq = spool.tile([J, K], dt.int32)
            nc.vector.tensor_scalar(out=xq[:], in0=yq[:], scalar1=W - 1,
                                    scalar2=None, op0=Alu.bitwise_and)
            xf = spool.tile([J, K], dt.float32)
            nc.vector.tensor_copy(xf[:], xq[:])
            hxc = spool.tile([J, K], dt.float32)
            hyc = spool.tile([J, K], dt.float32)
            nc.vector.tensor_tensor(out=hxc[:], in0=xf[:], in1=agg[:, K:2 * K],
                                    op=Alu.add)
            nc.vector.tensor_scalar_mul(hxc[:], hxc[:], float(stride))
            nc.vector.tensor_tensor(out=hyc[:], in0=yf[:], in1=agg[:, 2 * K:3 * K],
                                    op=Alu.add)
            nc.vector.tensor_scalar_mul(hyc[:], hyc[:], float(stride))
            cand_o = spool.tile([J, PT], dt.float32)
            nc.vector.memset(cand_o[:], 0.0)
            nc.vector.tensor_copy(cand_o[:, 0:K], hxc[:])
            nc.vector.tensor_copy(cand_o[:, K:2 * K], hyc[:])
            nc.vector.tensor_copy(cand_o[:, 2 * K:3 * K], agg[:, 0:K])
            nc.sync.dma_start(cand_d[:], cand_o[:])

            if debug:
                nc.sync.dma_start(dbg_d[:, 0:168], kall[:])
                nc.sync.dma_start(dbg_d[:, 168:336], gposall[:])
                nc.sync.dma_start(dbg_d[:, 336:368], fkeys[:])
                nc.sync.dma_start(dbg_d[:, 368:398], gposc[:])
                nc.sync.dma_start(dbg_d[:, 398:428], validm[:])
                nc.sync.dma_start(dbg_d[:, 428:444], key16[:])
                nc.sync.dma_start(dbg_d[:, 444:572], kpool2[:])
                nc.sync.dma_start(dbg_d[:, 572:573], cconst[:, 0:1])
                nc.sync.dma_start(dbg_d[:, 573:600], agg[:, 0:27])

            # ---------- score matrix W [36, 512] ----------
            cxf = spool.tile([1, JKP], dt.float32)
            cyf = spool.tile([1, JKP], dt.float32)
            nc.vector.memset(cxf[:], 0.0)
            nc.vector.memset(cyf[:], 0.0)
            nc.sync.dma_start(bass.AP(cxf.tensor, 0, [[JKP, 1], [1, JK]]), hxc[:])
            nc.sync.dma_start(bass.AP(cyf.tensor, 0, [[JKP, 1], [1, JK]]), hyc[:])
            cx2 = spool.tile([1, JKP], dt.float32)
            nc.vector.tensor_tensor(out=cx2[:], in0=cxf[:], in1=cxf[:], op=Alu.mult)
            cy2 = spool.tile([1, JKP], dt.float32)
            nc.vector.tensor_tensor(out=cy2[:], in0=cyf[:], in1=cyf[:], op=Alu.mult)
            c2 = spool.tile([1, JKP], dt.float32)
            nc.vector.tensor_tensor(out=c2[:], in0=cx2[:], in1=cy2[:], op=Alu.add)
            ones17 = spool.tile([1, J], dt.float32)
            nc.vector.memset(ones17[:], 1.0)
            cxm2 = spool.tile([1, JKP], dt.float32)
            nc.vector.tensor_scalar_mul(cxm2[:], cxf[:], -2.0)
            cym2 = spool.tile([1, JKP], dt.float32)
            nc.vector.tensor_scalar_mul(cym2[:], cyf[:], -2.0)
            wmat = spool.tile([CAUG, JKP], dt.float32)
            nc.vector.memset(wmat[:], 0.0)
            wx_ps = psB.tile([J, JKP], dt.float32, tag="psb")
            nc.tensor.matmul(wx_ps[:], ones17[:], cxm2[:], start=True, stop=True)
            nc.vector.tensor_tensor(out=wmat[0:J, :], in0=wx_ps[:], in1=mw[:],
                                    op=Alu.mult)
            wy_ps = psB.tile([J, JKP], dt.float32, tag="psb")
            nc.tensor.matmul(wy_ps[:], ones17[:], cym2[:], start=True, stop=True)
            wy_s = spool.tile([J, JKP], dt.float32)
            nc.vector.tensor_tensor(out=wy_s[:], in0=wy_ps[:], in1=mw[:], op=Alu.mult)
            nc.sync.dma_start(wmat[J:2 * J, :], wy_s[:])
            nc.sync.dma_start(wmat[34:35, :], c2[:])
            wmatc = spool.tile([CAUG, JKP], dt.float32)
            nc.vector.tensor_copy(wmatc[:], wmat[:])

            # ---------- gather table T (bf16 3-split) ----------
            cxP = spool.tile([PT, 4], dt.float32)
            cyP = spool.tile([PT, 4], dt.float32)
            for c in range(4):
                nc.sync.dma_start(cxP[:, c:c + 1],
                                  bass.AP(cxf.tensor, c * PT, [[JKP, 1], [1, PT]]))
                nc.sync.dma_start(cyP[:, c:c + 1],
                                  bass.AP(cyf.tensor, c * PT, [[JKP, 1], [1, PT]]))
            t_y = spool.tile([PT, 4 * 51], dt.float32)
            tfull = spool.tile([PT, 4 * 51], dt.float32)
            for c in range(4):
                nc.vector.scalar_tensor_tensor(
                    out=t_y[:, c * 51:(c + 1) * 51],
                    in0=mty[:, c * 51:(c + 1) * 51], scalar=cyP[:, c:c + 1],
                    in1=mtc[:, c * 51:(c + 1) * 51], op0=Alu.mult, op1=Alu.add)
                nc.vector.scalar_tensor_tensor(
                    out=tfull[:, c * 51:(c + 1) * 51],
                    in0=mtx[:, c * 51:(c + 1) * 51], scalar=cxP[:, c:c + 1],
                    in1=t_y[:, c * 51:(c + 1) * 51], op0=Alu.mult, op1=Alu.add)
            t_hi = spool.tile([PT, 4 * 51], dt.bfloat16)
            t_mid = spool.tile([PT, 4 * 51], dt.bfloat16)
            t_lo = spool.tile([PT, 4 * 51], dt.bfloat16)
            tr1 = spool.tile([PT, 4 * 51], dt.float32)
            tr1b = spool.tile([PT, 4 * 51], dt.float32)
            nc.vector.tensor_copy(t_hi[:], tfull[:])
            nc.vector.tensor_copy(tr1b[:], t_hi[:])
            nc.vector.tensor_tensor(out=tr1[:], in0=tfull[:], in1=tr1b[:],
                                    op=Alu.subtract)
            nc.vector.tensor_copy(t_mid[:], tr1[:])
            nc.vector.tensor_copy(tr1b[:], t_mid[:])
            nc.vector.tensor_tensor(out=tr1[:], in0=tr1[:], in1=tr1b[:],
                                    op=Alu.subtract)
            nc.vector.tensor_copy(t_lo[:], tr1[:])
            t_hi2 = spool.tile([PT, 4 * 51], dt.bfloat16)
            t_mid2 = spool.tile([PT, 4 * 51], dt.bfloat16)
            t_lo2 = spool.tile([PT, 4 * 51], dt.bfloat16)
            nc.scalar.copy(t_hi2[:], t_hi[:])
            nc.scalar.copy(t_mid2[:], t_mid[:])
            nc.scalar.copy(t_lo2[:], t_lo[:])

            # ---------- pose loop ----------
            for t in range(ntiles):
                ptile = posesb[:, t * CAUG:(t + 1) * CAUG]
                pT_ps = psB.tile([CAUG, PT], dt.float32, tag="ptps")
                nc.tensor.transpose(pT_ps[:], ptile, identfc[:])
                posesT = lpool.tile([CAUG, PT], dt.float32, tag="posesT")
                nc.vector.tensor_copy(posesT[:], pT_ps[:])
                score_ps = psA.tile([PT, JKP], dt.float32, tag="score")
                nc.tensor.matmul(score_ps[:], posesT[:], wmatc[:], start=True,
                                 stop=True)
                sc3 = bass.AP(score_ps.tensor, 0, [[JKP, PT], [K, J], [1, K]])
                rmin = lpool.tile([PT, J], dt.float32, tag="rmin")
                nc.vector.tensor_reduce(rmin[:], sc3, axis=Ax.X, op=Alu.min)
                rminp = lpool.tile([PT, J], dt.float32, tag="rminp")
                nc.vector.tensor_scalar_add(rminp[:], rmin[:], SCORE_THRESH)
                oh = lpool.tile([PT, JKP], dt.bfloat16, tag="oh")
                nc.vector.memset(oh[:, JK:JKP], 0.0)
                rb = bass.AP(rminp.tensor, 0, [[J, PT], [1, J], [0, K]])
                nc.vector.tensor_tensor(
                    out=bass.AP(oh.tensor, 0, [[JKP, PT], [K, J], [1, K]]),
                    in0=sc3, in1=rb, op=Alu.is_lt)
                ohT = lpool.tile([PT, JKP], dt.bfloat16, tag="ohT")
                for c in range(4):
                    ohT_ps = psA.tile([PT, PT], dt.bfloat16, tag="ohTps")
                    nc.tensor.transpose(ohT_ps[:], oh[:, c * PT:(c + 1) * PT],
                                        identbc[:])
                    nc.scalar.copy(ohT[:, c * PT:(c + 1) * PT], ohT_ps[:])
                g_ps = psA.tile([PT, 51], dt.float32, tag="gps")
                for c in range(4):
                    for si, s in enumerate((t_hi2, t_mid2, t_lo2)):
                        nc.tensor.matmul(g_ps[:], ohT[:, c * PT:(c + 1) * PT],
                                         s[:, c * 51:(c + 1) * 51],
                                         start=(c == 0 and si == 0),
                                         stop=(c == 3 and si == 2))
                gout = lpool.tile([PT, 51], dt.float32, tag="gout")
                nc.scalar.copy(gout[:], g_ps[:])
                nc.sync.dma_start(out_d[t * PT:(t + 1) * PT, :], gout[:])

    nc.compile()
    return nc


# --------------------------------------------------------------------------
# host-side constants / shards
# --------------------------------------------------------------------------
def _build_consts():
    import ml_dtypes
    c = {}
    c["identf"] = np.eye(PT, dtype=np.float32)
    c["identb"] = np.eye(PT, dtype=np.float32).astype(ml_dtypes.bfloat16)
    s = (np.arange(SW)[:, None] * RPC + np.arange(RPC)[None, :]).reshape(-1)
    c["revconst"] = np.broadcast_to((2047 - s).astype(np.float32),
                                    (PT, SLABF)).copy()
    mtx = np.zeros((PT, 4, 51), np.float32)
    mty = np.zeros((PT, 4, 51), np.float32)
    mtc = np.zeros((PT, 4, 51), np.float32)
    for ch in range(4):
        for p in range(PT):
            jk = ch * PT + p
            if jk < JK:
                j = jk // K
                mtx[p, ch, j] = 1.0
                mty[p, ch, 17 + j] = 1.0
                mtc[p, ch, 34 + j] = 1.0
    c["maskTx"] = mtx.reshape(PT, 4 * 51).copy()
    c["maskTy"] = mty.reshape(PT, 4 * 51).copy()
    c["maskTc"] = mtc.reshape(PT, 4 * 51).copy()
    mw = np.zeros((J, JKP), np.float32)
    for j in range(J):
        mw[j, j * K:(j + 1) * K] = 1.0
    c["maskW"] = mw
    jsel = np.zeros((PT, 8, J), np.float32)
    msel = np.zeros((PT, 8, 60), np.float32)
    for p in range(PT):
        for cs in range(8):
            i = p * 8 + cs
            if i < JK:
                jsel[p, cs, i // K] = 1.0
                msel[p, cs, i % K] = 1.0
            elif 512 <= i < 512 + JK:
                jsel[p, cs, (i - 512) // K] = 1.0
                msel[p, cs, 30 + (i - 512) % K] = 1.0
    c["jsel"] = jsel.reshape(PT, 8 * J).copy()
    c["msel"] = msel.reshape(PT, 8 * 60).copy()
    c["cgidx"] = np.broadcast_to(
        np.repeat(np.arange(NSLAB, dtype=np.float32), 8), (J, NSLAB * 8)).copy()
    return c


def _prep_shards(poses, heat, off):
    consts = _build_consts()
    heat_pad = np.full((J, H + 4, W + 4), -1.0, np.float32)
    heat_pad[:, 2:-2, 2:-2] = heat
    in_maps = []
    for core in range(NCORES):
        r0 = core * RPC
        lo = core * NPAD
        ps = poses[min(lo, len(poses)):min(lo + NPAD, len(poses))]
        pa = np.zeros((NPAD, CAUG), np.float32)
        if len(ps):
            pa[:len(ps), 0:17] = ps[:, 0::2]
            pa[:len(ps), 17:34] = ps[:, 1::2]
        pa[:, 34] = 1.0
        slab = np.full((NTILE_H * PT, SLABW, RW), -1.0, np.float32)
        for cg in range(NSLAB):
            tile_i, cg_l = divmod(cg, SPT)
            c0 = cg * SW
            ncol = min(SLABW, W + 4 - c0)
            blk = heat_pad[:, r0:r0 + RW, c0:c0 + ncol]       # [J, 68, ncol]
            for j in range(J):
                p = tile_i * PT + cg_l * J + j
                slab[p, :ncol, :] = blk[j].T
        m = {
            "poses": pa,
            "heat": slab.reshape(NTILE_H * PT, SLABW * RW),
            "offs": np.ascontiguousarray(
                off[:, :, r0:r0 + RPC, :]).reshape(OFFSZ, 1),
            "coreconst": np.broadcast_to(
                np.array([r0 * W, r0], np.float32), (J, 2)).copy(),
        }
        m.update(consts)
        in_maps.append(m)
    return in_maps


def _fixup(out_full, cnt, cand, poses):
    """Recompute sites where the one-hot matched != 1 candidate, exactly."""
    hx = cand[:, 0:K]
    hy = cand[:, K:2 * K]
    bad = np.argwhere(np.abs(cnt - 1.0) > 0.25)
    for n, j in bad:
        if n >= len(poses):
            continue
        px = np.float32(poses[n, 2 * j])
        py = np.float32(poses[n, 2 * j + 1])
        dx = (px - hx[j]).astype(np.float32)
        dy = (py - hy[j]).astype(np.float32)
        d2 = (dx * dx + dy * dy).astype(np.float32)
        kk = int(np.argmin(d2))
        out_full[n, 2 * j] = hx[j, kk]
        out_full[n, 2 * j + 1] = hy[j, kk]
    return out_full


def kernel(poses, heat_pred, offset_pred, stride):
    from concourse.bass_utils import run_bass_kernel_spmd

    poses = np.asarray(poses, dtype=np.float32)
    heat_pred = np.asarray(heat_pred, dtype=np.float32)
    offset_pred = np.asarray(offset_pred, dtype=np.float32)
    stride_v = int(np.asarray(stride).reshape(-1)[0]) if np.ndim(stride) else int(stride)

    key = ("prog", stride_v)
    if key not in _CACHE:
        _CACHE[key] = _build_program(stride_v)
    nc = _CACHE[key]

    in_maps = _prep_shards(poses, heat_pred, offset_pred)
    r = run_bass_kernel_spmd(nc, in_maps, list(range(NCORES)))
    global LAST_EXEC_NS
    LAST_EXEC_NS = r.exec_time_ns
    res = r.results

    outs = []
    cnts = []
    cand = np.asarray(res[0]["cand"], dtype=np.float32)
    for core in range(NCORES):
        o = np.asarray(res[core]["out"], dtype=np.float32)   # [NPAD, 51]
        outs.append(o)
    N = len(poses)
    full = np.zeros((N, 2 * J), np.float32)
    cnt_full = np.zeros((N, J), np.float32)
    for core in range(NCORES):
        lo = core * NPAD
        hi = min(lo + NPAD, N)
        if hi <= lo:
            break
        o = outs[core][:hi - lo]
        full[lo:hi, 0::2] = o[:, 0:17]
        full[lo:hi, 1::2] = o[:, 17:34]
        cnt_full[lo:hi] = o[:, 34:51]
    full = _fixup(full, cnt_full, cand, poses)
    return full

